# revision 42
# baseline (speedup 1.0000x reference)
"""Multi-head self-attention (B=2, N=2048, D=1024, H=16, Dh=64) on 8 TRN2 NeuronCores.

Sharding: core c handles batch b = c // 4 and head group g = c % 4 (heads 4g..4g+3).
Tensor-parallel on heads for qkv/out_proj; data-parallel on batch. Each core
produces a partial [D, N] output (transposed, fp16); host sums the 4 head-group
partials per batch, transposes, and adds b_out.

v3: single fused schedule built around the ScalarE (ACT) exp bottleneck
(~132us of exp per core). All matmul operands are bf16 (fp32 PSUM accum).
Emission order = Tile priority; the static scheduler weaves projection /
out-projection matmuls into PE idle slots of the ACT-bound attention stream.
  - dummy matmuls at t=0 warm the PE HAM clock (2.4 GHz) before real work,
  - q/k chains for pair 0 run interleaved per d-tile, tracking the x DMA,
  - v-projection chains interleave just-in-time into the first query block,
  - pair-1 q/k projection fills PE slack during B(p0), out-projection during
    B(p1); output written per 512-query chunk as one batched fp16 DMA.
Measured end-to-end relative error ~2e-3 (bf16 rounding), vs 2e-2 gate.
"""
import sys
import numpy as np

for _p in ("/opt/trn_rl_repo", "/root/.axon_site/_ro/trn_rl_repo"):
    if _p not in sys.path:
        sys.path.append(_p)

import ml_dtypes

import concourse.bass as bass
import concourse.bacc as bacc
import concourse.tile as tile
from concourse import mybir
from concourse.bass_utils import run_bass_kernel_spmd

F32 = mybir.dt.float32
BF16 = mybir.dt.bfloat16
F16 = mybir.dt.float16
EXP = mybir.ActivationFunctionType.Exp
ADD = mybir.AluOpType.add
MULT = mybir.AluOpType.mult

B, S, D = 2, 2048, 1024
H, DH = 16, 64
HL = 4            # heads per core (local); pairs p=0 (heads 0,1), p=1 (heads 2,3)
ND = D // 128     # 8 d-tiles (contraction)
NKT = S // 128    # 16 key tiles
NQC = S // 512    # 4 query chunks of 512
NWARM = 24        # dummy matmuls to warm the PE clock


def build_kernel() -> "bass.Bass":
    nc = bacc.Bacc(None, target_bir_lowering=False, debug=False)

    xT = nc.dram_tensor("xT", [D, S], BF16, kind="ExternalInput")
    wqk4 = nc.dram_tensor("wqk4", [128, 4, ND, 128], BF16, kind="ExternalInput")
    bqk = nc.dram_tensor("bqk", [128, 4], F32, kind="ExternalInput")
    wvp = nc.dram_tensor("wvp", [128, ND, 256], BF16, kind="ExternalInput")
    bvb = nc.dram_tensor("bvb", [128, 256], F32, kind="ExternalInput")
    woutp = nc.dram_tensor("woutp", [128, 2, D], BF16, kind="ExternalInput")
    # output in [partition, d-tile, seq] layout so each 512-query chunk is
    # one strided DMA; host reassembles [D, S]
    outP = nc.dram_tensor("outP", [128, ND, S], F16, kind="ExternalOutput")

    xT_r = xT.rearrange("(t p) s -> t p s", p=128)        # [8, 128, 2048]

    with tile.TileContext(nc) as tc:
        with tc.tile_pool(name="persist", bufs=1) as persist, \
             tc.tile_pool(name="ptp", bufs=16) as ptp, \
             tc.tile_pool(name="small", bufs=3) as small, \
             tc.tile_pool(name="stage", bufs=2) as stage, \
             tc.tile_pool(name="score_ps", bufs=2, space="PSUM") as psb, \
             tc.tile_pool(name="av_ps", bufs=1, space="PSUM") as psav, \
             tc.tile_pool(name="fill_ps", bufs=2, space="PSUM") as fill:

            xt_s = persist.tile([128, ND, S], BF16)
            wqk_s = persist.tile([128, 4, ND, 128], BF16)
            wv_s = persist.tile([128, ND, 256], BF16)
            wout_s = persist.tile([128, 2, D], BF16)
            bqk_s = persist.tile([128, 4], F32)
            bvb_s = persist.tile([128, 256], F32)
            qkt_s = persist.tile([128, 4, S], BF16)   # m: 0=q_p0 1=k_p0 2=q_p1 3=k_p1
            v_s = persist.tile([128, NKT, HL, DH + 1], BF16)
            at_s = persist.tile([128, 2, S], BF16)
            warm_s = persist.tile([128, 512], BF16)

            # ---------------- input DMAs (order = arrival order) ----------
            nc.sync.dma_start(out=xt_s[:, 0], in_=xT_r[0])
            nc.sync.dma_start(out=wqk_s[:, 0], in_=wqk4[:, 0])
            nc.sync.dma_start(out=wqk_s[:, 1], in_=wqk4[:, 1])
            nc.sync.dma_start(out=bqk_s[:], in_=bqk[:])
            nc.sync.dma_start(out=bvb_s[:], in_=bvb[:])
            for d in range(1, ND):
                nc.sync.dma_start(out=xt_s[:, d], in_=xT_r[d])
            nc.sync.dma_start(out=wv_s[:], in_=wvp[:])
            nc.sync.dma_start(out=wqk_s[:, 2], in_=wqk4[:, 2])
            nc.sync.dma_start(out=wqk_s[:, 3], in_=wqk4[:, 3])
            nc.sync.dma_start(out=wout_s[:], in_=woutp[:])
            nc.vector.memset(v_s[:, :, :, DH:DH + 1], 1.0)

            # ---------------- PE warm-up: dep-free dummy matmuls ----------
            nc.vector.memset(warm_s[:], 0.0)
            wps = fill.tile([128, 512], F32, tag="fill", name="warm")
            for i in range(NWARM):
                nc.tensor.matmul(wps[:], warm_s[:, 0:128], warm_s[:],
                                 start=(i % 8 == 0),
                                 stop=(i % 8 == 7 or i == NWARM - 1))

            def bias_add(m, n):
                nc.vector.tensor_scalar_add(
                    qkt_s[:, m, n * 512:(n + 1) * 512], _chain_ps[(m, n)][:],
                    bqk_s[:, m:m + 1])

            _chain_ps = {}

            def proj_chain(m, n, pool=None, tag=None, interleave=False):
                """qkt_s[:, m, n*512:...] = wqk[:, m].T @ x chunk + bias."""
                p_ = pool or fill
                if tag is None:
                    tag = "sAB" if p_ is psb else "fill"
                ps = p_.tile([128, 512], F32, tag=tag, name=f"a1_{m}_{n}")
                _chain_ps[(m, n)] = ps
                if interleave:
                    return  # MMs emitted by the caller per-d
                for d in range(ND):
                    nc.tensor.matmul(ps[:], wqk_s[:, m, d, :],
                                     xt_s[:, d, n * 512:(n + 1) * 512],
                                     start=(d == 0), stop=(d == ND - 1))
                bias_add(m, n)

            _a2_ps = {}

            def a2_chain(st, pool=None, tag=None, interleave=False):
                """v_s[:, st] = x tile st @ wv + bias (keypos-partition layout)."""
                p_ = pool or fill
                ps = p_.tile([128, 256], F32, tag=tag or "fill", name=f"a2_{st}")
                _a2_ps[st] = ps
                if interleave:
                    return  # MMs emitted by the caller per-d
                for d in range(ND):
                    nc.tensor.matmul(ps[:], xt_s[:, d, st * 128:(st + 1) * 128],
                                     wv_s[:, d, :],
                                     start=(d == 0), stop=(d == ND - 1))
                a2_bias(st)

            def a2_bias(st):
                nc.vector.tensor_tensor(
                    out=v_s[:, st, :, 0:DH],
                    in0=_a2_ps[st][:].rearrange("p (h c) -> p h c", h=HL),
                    in1=bvb_s[:].rearrange("p (h c) -> p h c", h=HL),
                    op=ADD)

            def normalize(p, qb, pX, loc):
                """at_s[64*loc:..., p, qb] = pX[0:64] / pX[64] (softmax denom)."""
                qs = slice(qb * 512, qb * 512 + 512)
                # the DMA reshape to [64,8] lets the reciprocal run 64-wide
                # (a [1,512] reciprocal is ~3.3us of serial DVE and backlogs
                # the engine at every block boundary)
                raw = small.tile([DH + 1, 512], F32, tag="raw", name="raw")
                nc.vector.tensor_copy(out=raw[:], in_=pX[:])
                dn = small.tile([64, 8], F32, tag="dn", name="dn")
                nc.sync.dma_start(out=dn[:], in_=raw[DH:DH + 1, :])
                rr = small.tile([64, 8], F32, tag="rr", name="rr")
                nc.vector.reciprocal(rr[:], dn[:])
                r = small.tile([1, 512], F32, tag="r", name="r")
                nc.sync.dma_start(out=r[:], in_=rr[:])
                rb = small.tile([64, 512], F32, tag="rb", name="rb")
                nc.gpsimd.partition_broadcast(rb[:], r[:])
                if loc == 0:
                    nc.vector.tensor_tensor(
                        out=at_s[0:64, p, qs], in0=raw[0:DH, :], in1=rb[:],
                        op=MULT)
                else:
                    # DVE lanes cannot shift partitions; bounce via DMA
                    tmp = small.tile([64, 512], BF16, tag="tmp", name="tmp")
                    nc.vector.tensor_tensor(
                        out=tmp[:], in0=raw[0:DH, :], in1=rb[:], op=MULT)
                    nc.sync.dma_start(out=at_s[64:128, p, qs], in_=tmp[:])

            def b_block(p, qb, a2_from=None, fillers=None):
                qs = slice(qb * 512, qb * 512 + 512)
                qt = qkt_s[:, 2 * p, :]
                kt = qkt_s[:, 2 * p + 1, :]
                pA = psav.tile([DH + 1, 512], F32, tag="pA", name="pA")
                pB = psav.tile([DH + 1, 512], F32, tag="pB", name="pB")
                pts = {}

                def av(t):
                    nc.tensor.matmul(pA[:], v_s[:, t, 2 * p, :],
                                     pts[t][:, 0:512],
                                     start=(t == 0), stop=(t == NKT - 1))
                    nc.tensor.matmul(pB[:], v_s[:, t, 2 * p + 1, :],
                                     pts[t][:, 512:1024],
                                     start=(t == 0), stop=(t == NKT - 1))

                for t in range(NKT):
                    if fillers and t in fillers:
                        fillers[t]()
                    if a2_from is not None and t >= a2_from:
                        a2_chain(t)
                    tcols = slice(t * 128, (t + 1) * 128)
                    sAB = psb.tile([128, 1024], F32, tag="sAB", name="sAB")
                    nc.tensor.matmul(sAB[:, 0:512], kt[0:64, tcols],
                                     qt[0:64, qs], start=True, stop=True,
                                     tile_position=(0, 0))
                    nc.tensor.matmul(sAB[:, 512:1024], kt[64:128, tcols],
                                     qt[64:128, qs], start=True, stop=True,
                                     tile_position=(64, 0))
                    pt = ptp.tile([128, 1024], BF16, tag="pt", name="pt")
                    pts[t] = pt
                    nc.scalar.activation(pt[:], sAB[:], EXP)
                    # AV pipelined one t behind: its wait on exp(t-1) is then
                    # already satisfied, keeping sem latency off the PE stream
                    if t > 0:
                        av(t - 1)
                av(NKT - 1)
                normalize(p, qb, pA, 0)
                normalize(p, qb, pB, 1)

            def c_block(qb, nsplit=1):
                qs = slice(qb * 512, qb * 512 + 512)
                ob = stage.tile([128, ND, 512], F16, tag="o", name=f"o{qb}")
                step = ND // nsplit
                for nt in range(ND):
                    po = fill.tile([128, 512], F32, tag="fill",
                                   name=f"c_{qb}_{nt}")
                    nc.tensor.matmul(po[:], wout_s[:, 0, nt * 128:(nt + 1) * 128],
                                     at_s[:, 0, qs], start=True, stop=False)
                    nc.tensor.matmul(po[:], wout_s[:, 1, nt * 128:(nt + 1) * 128],
                                     at_s[:, 1, qs], start=False, stop=True)
                    nc.vector.tensor_copy(out=ob[:, nt, :], in_=po[:])
                    if (nt + 1) % step == 0:
                        lo = nt + 1 - step
                        nc.sync.dma_start(out=outP[:, lo:nt + 1, qs],
                                          in_=ob[:, lo:nt + 1, :])

            # ---------------- emission order = scheduler priority ----------
            # pair-0 q (first chunk) + pair-0 k chunks n0..n2, interleaved
            # per d-tile so the chains track the x DMA arrival
            # all pair-0 k chunks + first q chunk track the x DMA per d-tile;
            # m1n3 borrows the pA psum slot so nothing serializes between
            # x-complete and the first scores
            prologue = [(0, 0, psb, None), (1, 0, psb, None),
                        (1, 1, fill, None), (1, 2, fill, None),
                        (1, 3, psav, "pA")]
            for m, n, pool, tag in prologue:
                proj_chain(m, n, pool=pool, tag=tag, interleave=True)
            a2_chain(0, pool=psav, tag="pB", interleave=True)
            for d in range(ND):
                for m, n, pool, tag in prologue:
                    nc.tensor.matmul(_chain_ps[(m, n)][:], wqk_s[:, m, d, :],
                                     xt_s[:, d, n * 512:(n + 1) * 512],
                                     start=(d == 0), stop=(d == ND - 1))
                nc.tensor.matmul(_a2_ps[0][:], xt_s[:, d, 0:128],
                                 wv_s[:, d, :],
                                 start=(d == 0), stop=(d == ND - 1))
            for m, n, pool, tag in prologue:
                bias_add(m, n)
            a2_bias(0)
            proj_chain(0, 1)            # q chunk for qb1: if left until the
            # qb0/qb1 boundary it becomes a ~3us ACT gap (qb0 has no PE slack)
            # B(p0): qb0 carries the remaining v-projection chains just-in-time
            b_block(0, 0, a2_from=1)
            for qb in range(1, NQC):
                if qb > 1:
                    proj_chain(0, qb)   # q chunk for this qb
                b_block(0, qb)
            # pair-1 q/k projections: gap-fill B(p0) slack / run before B(p1).
            # k chunk 0 + q chunk 0 first — they alone gate B(p1,qb0)'s start
            for m, n in ((3, 0), (2, 0), (3, 1), (3, 2), (3, 3),
                         (2, 1), (2, 2), (2, 3)):
                proj_chain(m, n)
            for qb in range(NQC):
                b_block(1, qb)
            # out-projection: gap-fills B(p1) slack, tail for last chunk
            # (written in 4 sub-DMAs so the final transfer starts early)
            for qb in range(NQC):
                c_block(qb, nsplit=1 if qb < NQC - 1 else 4)
            # keep the PE clock warm through the final normalize latency so
            # the last out-projection matmuls don't run at K=4/8
            wps2 = psb.tile([128, 512], F32, tag="sAB", name="warm2")
            for i in range(10):
                nc.tensor.matmul(wps2[:], warm_s[:, 0:128], warm_s[:],
                                 start=(i == 0), stop=(i == 9))
    nc.compile()
    return nc


def shard_inputs(x, W_qkv, b_qkv, W_out, b_out=None):
    """Build the 8 per-core input maps. Core c: batch c//4, head group c%4."""
    in_maps = []
    scale = 1.0 / np.sqrt(np.float32(DH))
    bf16 = ml_dtypes.bfloat16
    for c in range(8):
        b, g = divmod(c, 4)
        cs = slice(g * 256, g * 256 + 256)
        xTc = np.ascontiguousarray(x[b].T).astype(bf16)          # [D, S]
        wq = W_qkv[:, 0:D][:, cs] * scale                        # [D, 256]
        wk = W_qkv[:, D:2 * D][:, cs]
        # m-groups: 0=q_p0 1=k_p0 2=q_p1 3=k_p1
        wm = np.stack([wq[:, 0:128], wk[:, 0:128],
                       wq[:, 128:256], wk[:, 128:256]], axis=0)  # [4, D, 128]
        wqk4 = np.ascontiguousarray(
            wm.reshape(4, ND, 128, 128).transpose(2, 0, 1, 3)).astype(bf16)
        bq = b_qkv[0:D][cs] * scale
        bk = b_qkv[D:2 * D][cs]
        bqkc = np.ascontiguousarray(np.stack(
            [bq[0:128], bk[0:128], bq[128:256], bk[128:256]],
            axis=1)).astype(np.float32)                          # [128, 4]
        wv = W_qkv[:, 2 * D:3 * D][:, cs]                        # [D, 256]
        wvp = np.ascontiguousarray(
            wv.reshape(ND, 128, 256).transpose(1, 0, 2)).astype(bf16)
        bvbc = np.ascontiguousarray(
            np.broadcast_to(b_qkv[2 * D:3 * D][cs], (128, 256))).astype(np.float32)
        wo = W_out[cs, :]                                        # [256, D]
        woutp = np.ascontiguousarray(
            wo.reshape(2, 128, D).transpose(1, 0, 2)).astype(bf16)
        in_maps.append({
            "xT": xTc,
            "wqk4": wqk4,
            "bqk": bqkc,
            "wvp": wvp,
            "bvb": bvbc,
            "woutp": woutp,
        })
    return in_maps


_NC_CACHE = []


def _get_nc():
    if not _NC_CACHE:
        _NC_CACHE.append(build_kernel())
    return _NC_CACHE[0]


def run_sharded(in_maps, **kwargs):
    nc = _get_nc()
    return run_bass_kernel_spmd(nc, in_maps, core_ids=list(range(8)), **kwargs)


def gather_output(results, b_out):
    out = np.empty((B, S, D), dtype=np.float32)
    for b in range(B):
        # outP [128, ND, S] -> [D, S]: row d*128+p = outP[p, d]
        acc = results[4 * b]["outP"].astype(np.float32)
        for g in range(1, 4):
            acc = acc + results[4 * b + g]["outP"].astype(np.float32)
        outT = acc.transpose(1, 0, 2).reshape(D, S)
        out[b] = outT.T + b_out[None, :]
    return out


def kernel(x, W_qkv, b_qkv, W_out, b_out):
    x = np.asarray(x, dtype=np.float32)
    W_qkv = np.asarray(W_qkv, dtype=np.float32)
    b_qkv = np.asarray(b_qkv, dtype=np.float32)
    W_out = np.asarray(W_out, dtype=np.float32)
    b_out = np.asarray(b_out, dtype=np.float32)
    in_maps = shard_inputs(x=x, W_qkv=W_qkv, b_qkv=b_qkv, W_out=W_out, b_out=b_out)
    res = run_sharded(in_maps)
    return gather_output(res.results, b_out)


# revision 43
# speedup vs baseline: 1.0041x; 1.0041x over previous
"""Multi-head self-attention (B=2, N=2048, D=1024, H=16, Dh=64) on 8 TRN2 NeuronCores.

Sharding: core c handles batch b = c // 4 and head group g = c % 4 (heads 4g..4g+3).
Tensor-parallel on heads for qkv/out_proj; data-parallel on batch. Each core
produces a partial [D, N] output (transposed, fp16); host sums the 4 head-group
partials per batch, transposes, and adds b_out.

v3: single fused schedule built around the ScalarE (ACT) exp bottleneck
(~132us of exp per core). All matmul operands are bf16 (fp32 PSUM accum).
Emission order = Tile priority; the static scheduler weaves projection /
out-projection matmuls into PE idle slots of the ACT-bound attention stream.
  - dummy matmuls at t=0 warm the PE HAM clock (2.4 GHz) before real work,
  - q/k chains for pair 0 run interleaved per d-tile, tracking the x DMA,
  - v-projection chains interleave just-in-time into the first query block,
  - pair-1 q/k projection fills PE slack during B(p0), out-projection during
    B(p1); output written per 512-query chunk as one batched fp16 DMA.
Measured end-to-end relative error ~2e-3 (bf16 rounding), vs 2e-2 gate.
"""
import sys
import numpy as np

for _p in ("/opt/trn_rl_repo", "/root/.axon_site/_ro/trn_rl_repo"):
    if _p not in sys.path:
        sys.path.append(_p)

import ml_dtypes

import concourse.bass as bass
import concourse.bacc as bacc
import concourse.tile as tile
from concourse import mybir
from concourse.bass_utils import run_bass_kernel_spmd

F32 = mybir.dt.float32
BF16 = mybir.dt.bfloat16
F16 = mybir.dt.float16
EXP = mybir.ActivationFunctionType.Exp
ADD = mybir.AluOpType.add
MULT = mybir.AluOpType.mult

B, S, D = 2, 2048, 1024
H, DH = 16, 64
HL = 4            # heads per core (local); pairs p=0 (heads 0,1), p=1 (heads 2,3)
ND = D // 128     # 8 d-tiles (contraction)
NKT = S // 128    # 16 key tiles
NQC = S // 512    # 4 query chunks of 512
NWARM = 24        # dummy matmuls to warm the PE clock


def build_kernel() -> "bass.Bass":
    nc = bacc.Bacc(None, target_bir_lowering=False, debug=False)

    xT = nc.dram_tensor("xT", [D, S], BF16, kind="ExternalInput")
    wqk4 = nc.dram_tensor("wqk4", [128, 4, ND, 128], BF16, kind="ExternalInput")
    bqk = nc.dram_tensor("bqk", [128, 4], F32, kind="ExternalInput")
    wvp = nc.dram_tensor("wvp", [128, ND, 256], BF16, kind="ExternalInput")
    bvb = nc.dram_tensor("bvb", [128, 256], F32, kind="ExternalInput")
    woutp = nc.dram_tensor("woutp", [128, 2, D], BF16, kind="ExternalInput")
    # output in [partition, d-tile, seq] layout so each 512-query chunk is
    # one strided DMA; host reassembles [D, S]
    outP = nc.dram_tensor("outP", [128, ND, S], F16, kind="ExternalOutput")

    xT_r = xT.rearrange("(t p) s -> t p s", p=128)        # [8, 128, 2048]

    with tile.TileContext(nc) as tc:
        with tc.tile_pool(name="persist", bufs=1) as persist, \
             tc.tile_pool(name="ptp", bufs=16) as ptp, \
             tc.tile_pool(name="small", bufs=3) as small, \
             tc.tile_pool(name="stage", bufs=2) as stage, \
             tc.tile_pool(name="score_ps", bufs=2, space="PSUM") as psb, \
             tc.tile_pool(name="av_ps", bufs=1, space="PSUM") as psav, \
             tc.tile_pool(name="fill_ps", bufs=2, space="PSUM") as fill:

            xt_s = persist.tile([128, ND, S], BF16)
            wqk_s = persist.tile([128, 4, ND, 128], BF16)
            wv_s = persist.tile([128, ND, 256], BF16)
            wout_s = persist.tile([128, 2, D], BF16)
            bqk_s = persist.tile([128, 4], F32)
            bvb_s = persist.tile([128, 256], F32)
            qkt_s = persist.tile([128, 4, S], BF16)   # m: 0=q_p0 1=k_p0 2=q_p1 3=k_p1
            v_s = persist.tile([128, NKT, HL, DH + 1], BF16)
            at_s = persist.tile([128, 2, S], BF16)
            warm_s = persist.tile([128, 512], BF16)

            # ---------------- input DMAs (order = arrival order) ----------
            nc.sync.dma_start(out=xt_s[:, 0], in_=xT_r[0])
            nc.sync.dma_start(out=wqk_s[:, 0], in_=wqk4[:, 0])
            nc.sync.dma_start(out=wqk_s[:, 1], in_=wqk4[:, 1])
            nc.sync.dma_start(out=bqk_s[:], in_=bqk[:])
            nc.sync.dma_start(out=bvb_s[:], in_=bvb[:])
            for d in range(1, ND):
                nc.sync.dma_start(out=xt_s[:, d], in_=xT_r[d])
            nc.sync.dma_start(out=wv_s[:], in_=wvp[:])
            nc.sync.dma_start(out=wqk_s[:, 2], in_=wqk4[:, 2])
            nc.sync.dma_start(out=wqk_s[:, 3], in_=wqk4[:, 3])
            nc.sync.dma_start(out=wout_s[:], in_=woutp[:])
            nc.vector.memset(v_s[:, :, :, DH:DH + 1], 1.0)

            # ---------------- PE warm-up: dep-free dummy matmuls ----------
            nc.vector.memset(warm_s[:], 0.0)
            wps = fill.tile([128, 512], F32, tag="fill", name="warm")
            for i in range(NWARM):
                nc.tensor.matmul(wps[:], warm_s[:, 0:128], warm_s[:],
                                 start=(i % 8 == 0),
                                 stop=(i % 8 == 7 or i == NWARM - 1))

            def bias_add(m, n):
                nc.vector.tensor_scalar_add(
                    qkt_s[:, m, n * 512:(n + 1) * 512], _chain_ps[(m, n)][:],
                    bqk_s[:, m:m + 1])

            _chain_ps = {}

            def proj_chain(m, n, pool=None, tag=None, interleave=False):
                """qkt_s[:, m, n*512:...] = wqk[:, m].T @ x chunk + bias."""
                p_ = pool or fill
                if tag is None:
                    tag = "sAB" if p_ is psb else "fill"
                ps = p_.tile([128, 512], F32, tag=tag, name=f"a1_{m}_{n}")
                _chain_ps[(m, n)] = ps
                if interleave:
                    return  # MMs emitted by the caller per-d
                for d in range(ND):
                    nc.tensor.matmul(ps[:], wqk_s[:, m, d, :],
                                     xt_s[:, d, n * 512:(n + 1) * 512],
                                     start=(d == 0), stop=(d == ND - 1))
                bias_add(m, n)

            _a2_ps = {}

            def a2_chain(st, pool=None, tag=None, interleave=False):
                """v_s[:, st] = x tile st @ wv + bias (keypos-partition layout)."""
                p_ = pool or fill
                ps = p_.tile([128, 256], F32, tag=tag or "fill", name=f"a2_{st}")
                _a2_ps[st] = ps
                if interleave:
                    return  # MMs emitted by the caller per-d
                for d in range(ND):
                    nc.tensor.matmul(ps[:], xt_s[:, d, st * 128:(st + 1) * 128],
                                     wv_s[:, d, :],
                                     start=(d == 0), stop=(d == ND - 1))
                a2_bias(st)

            def a2_bias(st):
                nc.vector.tensor_tensor(
                    out=v_s[:, st, :, 0:DH],
                    in0=_a2_ps[st][:].rearrange("p (h c) -> p h c", h=HL),
                    in1=bvb_s[:].rearrange("p (h c) -> p h c", h=HL),
                    op=ADD)

            def normalize(p, qb, pX, loc):
                """at_s[64*loc:..., p, qb] = pX[0:64] / pX[64] (softmax denom)."""
                qs = slice(qb * 512, qb * 512 + 512)
                # the DMA reshape to [64,8] lets the reciprocal run 64-wide
                # (a [1,512] reciprocal is ~3.3us of serial DVE and backlogs
                # the engine at every block boundary)
                raw = small.tile([DH + 1, 512], F32, tag="raw", name="raw")
                nc.vector.tensor_copy(out=raw[:], in_=pX[:])
                dn = small.tile([64, 8], F32, tag="dn", name="dn")
                nc.sync.dma_start(out=dn[:], in_=raw[DH:DH + 1, :])
                rr = small.tile([64, 8], F32, tag="rr", name="rr")
                nc.vector.reciprocal(rr[:], dn[:])
                r = small.tile([1, 512], F32, tag="r", name="r")
                nc.sync.dma_start(out=r[:], in_=rr[:])
                rb = small.tile([64, 512], F32, tag="rb", name="rb")
                nc.gpsimd.partition_broadcast(rb[:], r[:])
                if loc == 0:
                    nc.vector.tensor_tensor(
                        out=at_s[0:64, p, qs], in0=raw[0:DH, :], in1=rb[:],
                        op=MULT)
                else:
                    # DVE lanes cannot shift partitions; bounce via DMA
                    tmp = small.tile([64, 512], BF16, tag="tmp", name="tmp")
                    nc.vector.tensor_tensor(
                        out=tmp[:], in0=raw[0:DH, :], in1=rb[:], op=MULT)
                    nc.sync.dma_start(out=at_s[64:128, p, qs], in_=tmp[:])

            def b_block(p, qb, a2_from=None, fillers=None):
                qs = slice(qb * 512, qb * 512 + 512)
                qt = qkt_s[:, 2 * p, :]
                kt = qkt_s[:, 2 * p + 1, :]
                pA = psav.tile([DH + 1, 512], F32, tag="pA", name="pA")
                pB = psav.tile([DH + 1, 512], F32, tag="pB", name="pB")
                pts = {}

                def av(t):
                    nc.tensor.matmul(pA[:], v_s[:, t, 2 * p, :],
                                     pts[t][:, 0:512],
                                     start=(t == 0), stop=(t == NKT - 1))
                    nc.tensor.matmul(pB[:], v_s[:, t, 2 * p + 1, :],
                                     pts[t][:, 512:1024],
                                     start=(t == 0), stop=(t == NKT - 1))

                for t in range(NKT):
                    if fillers and t in fillers:
                        fillers[t]()
                    if a2_from is not None and t >= a2_from:
                        a2_chain(t)
                    tcols = slice(t * 128, (t + 1) * 128)
                    sAB = psb.tile([128, 1024], F32, tag="sAB", name="sAB")
                    nc.tensor.matmul(sAB[:, 0:512], kt[0:64, tcols],
                                     qt[0:64, qs], start=True, stop=True,
                                     tile_position=(0, 0))
                    nc.tensor.matmul(sAB[:, 512:1024], kt[64:128, tcols],
                                     qt[64:128, qs], start=True, stop=True,
                                     tile_position=(64, 0))
                    pt = ptp.tile([128, 1024], BF16, tag="pt", name="pt")
                    pts[t] = pt
                    nc.scalar.activation(pt[:], sAB[:], EXP)
                    # AV pipelined one t behind: its wait on exp(t-1) is then
                    # already satisfied, keeping sem latency off the PE stream
                    if t > 0:
                        av(t - 1)
                av(NKT - 1)
                normalize(p, qb, pA, 0)
                normalize(p, qb, pB, 1)

            def c_block(qb, nsplit=1):
                qs = slice(qb * 512, qb * 512 + 512)
                ob = stage.tile([128, ND, 512], F16, tag="o", name=f"o{qb}")
                step = ND // nsplit
                for nt in range(ND):
                    po = fill.tile([128, 512], F32, tag="fill",
                                   name=f"c_{qb}_{nt}")
                    nc.tensor.matmul(po[:], wout_s[:, 0, nt * 128:(nt + 1) * 128],
                                     at_s[:, 0, qs], start=True, stop=False)
                    nc.tensor.matmul(po[:], wout_s[:, 1, nt * 128:(nt + 1) * 128],
                                     at_s[:, 1, qs], start=False, stop=True)
                    nc.vector.tensor_copy(out=ob[:, nt, :], in_=po[:])
                    if (nt + 1) % step == 0:
                        lo = nt + 1 - step
                        nc.sync.dma_start(out=outP[:, lo:nt + 1, qs],
                                          in_=ob[:, lo:nt + 1, :])

            # ---------------- emission order = scheduler priority ----------
            # pair-0 q (first chunk) + pair-0 k chunks n0..n2, interleaved
            # per d-tile so the chains track the x DMA arrival
            prologue = [(0, 0, psb, None), (1, 0, psb, None),
                        (1, 1, fill, None), (1, 2, fill, None)]
            for m, n, pool, tag in prologue:
                proj_chain(m, n, pool=pool, tag=tag, interleave=True)
            a2_chain(0, pool=psav, tag="pA", interleave=True)
            a2_chain(1, pool=psav, tag="pB", interleave=True)
            for d in range(ND):
                for m, n, pool, tag in prologue:
                    nc.tensor.matmul(_chain_ps[(m, n)][:], wqk_s[:, m, d, :],
                                     xt_s[:, d, n * 512:(n + 1) * 512],
                                     start=(d == 0), stop=(d == ND - 1))
                for st in (0, 1):
                    nc.tensor.matmul(_a2_ps[st][:],
                                     xt_s[:, d, st * 128:(st + 1) * 128],
                                     wv_s[:, d, :],
                                     start=(d == 0), stop=(d == ND - 1))
            for m, n, pool, tag in prologue:
                bias_add(m, n)
            for st in (0, 1):
                a2_bias(st)
            proj_chain(1, 3)
            proj_chain(0, 1)            # q chunk for qb1: if left until the
            # qb0/qb1 boundary it becomes a ~3us ACT gap (qb0 has no PE slack)
            # B(p0): qb0 carries the remaining v-projection chains just-in-time
            b_block(0, 0, a2_from=2)
            for qb in range(1, NQC):
                if qb > 1:
                    proj_chain(0, qb)   # q chunk for this qb
                b_block(0, qb)
            # pair-1 q/k projections: gap-fill B(p0) slack / run before B(p1).
            # k chunk 0 + q chunk 0 first — they alone gate B(p1,qb0)'s start
            for m, n in ((3, 0), (2, 0), (3, 1), (3, 2), (3, 3),
                         (2, 1), (2, 2), (2, 3)):
                proj_chain(m, n)
            for qb in range(NQC):
                b_block(1, qb)
            # out-projection: gap-fills B(p1) slack, tail for last chunk
            # (written in 4 sub-DMAs so the final transfer starts early)
            for qb in range(NQC):
                c_block(qb, nsplit=1 if qb < NQC - 1 else 4)
            # keep the PE clock warm through the final normalize latency so
            # the last out-projection matmuls don't run at K=4/8
            wps2 = psb.tile([128, 512], F32, tag="sAB", name="warm2")
            for i in range(10):
                nc.tensor.matmul(wps2[:], warm_s[:, 0:128], warm_s[:],
                                 start=(i == 0), stop=(i == 9))
    nc.compile()
    return nc


def shard_inputs(x, W_qkv, b_qkv, W_out, b_out=None):
    """Build the 8 per-core input maps. Core c: batch c//4, head group c%4."""
    in_maps = []
    scale = 1.0 / np.sqrt(np.float32(DH))
    bf16 = ml_dtypes.bfloat16
    for c in range(8):
        b, g = divmod(c, 4)
        cs = slice(g * 256, g * 256 + 256)
        xTc = np.ascontiguousarray(x[b].T).astype(bf16)          # [D, S]
        wq = W_qkv[:, 0:D][:, cs] * scale                        # [D, 256]
        wk = W_qkv[:, D:2 * D][:, cs]
        # m-groups: 0=q_p0 1=k_p0 2=q_p1 3=k_p1
        wm = np.stack([wq[:, 0:128], wk[:, 0:128],
                       wq[:, 128:256], wk[:, 128:256]], axis=0)  # [4, D, 128]
        wqk4 = np.ascontiguousarray(
            wm.reshape(4, ND, 128, 128).transpose(2, 0, 1, 3)).astype(bf16)
        bq = b_qkv[0:D][cs] * scale
        bk = b_qkv[D:2 * D][cs]
        bqkc = np.ascontiguousarray(np.stack(
            [bq[0:128], bk[0:128], bq[128:256], bk[128:256]],
            axis=1)).astype(np.float32)                          # [128, 4]
        wv = W_qkv[:, 2 * D:3 * D][:, cs]                        # [D, 256]
        wvp = np.ascontiguousarray(
            wv.reshape(ND, 128, 256).transpose(1, 0, 2)).astype(bf16)
        bvbc = np.ascontiguousarray(
            np.broadcast_to(b_qkv[2 * D:3 * D][cs], (128, 256))).astype(np.float32)
        wo = W_out[cs, :]                                        # [256, D]
        woutp = np.ascontiguousarray(
            wo.reshape(2, 128, D).transpose(1, 0, 2)).astype(bf16)
        in_maps.append({
            "xT": xTc,
            "wqk4": wqk4,
            "bqk": bqkc,
            "wvp": wvp,
            "bvb": bvbc,
            "woutp": woutp,
        })
    return in_maps


_NC_CACHE = []


def _get_nc():
    if not _NC_CACHE:
        _NC_CACHE.append(build_kernel())
    return _NC_CACHE[0]


def run_sharded(in_maps, **kwargs):
    nc = _get_nc()
    return run_bass_kernel_spmd(nc, in_maps, core_ids=list(range(8)), **kwargs)


def gather_output(results, b_out):
    out = np.empty((B, S, D), dtype=np.float32)
    for b in range(B):
        # outP [128, ND, S] -> [D, S]: row d*128+p = outP[p, d]
        acc = results[4 * b]["outP"].astype(np.float32)
        for g in range(1, 4):
            acc = acc + results[4 * b + g]["outP"].astype(np.float32)
        outT = acc.transpose(1, 0, 2).reshape(D, S)
        out[b] = outT.T + b_out[None, :]
    return out


def kernel(x, W_qkv, b_qkv, W_out, b_out):
    x = np.asarray(x, dtype=np.float32)
    W_qkv = np.asarray(W_qkv, dtype=np.float32)
    b_qkv = np.asarray(b_qkv, dtype=np.float32)
    W_out = np.asarray(W_out, dtype=np.float32)
    b_out = np.asarray(b_out, dtype=np.float32)
    in_maps = shard_inputs(x=x, W_qkv=W_qkv, b_qkv=b_qkv, W_out=W_out, b_out=b_out)
    res = run_sharded(in_maps)
    return gather_output(res.results, b_out)
